# revision 1
# baseline (speedup 1.0000x reference)
"""AttentionLSTM Trainium2 kernel: data-parallel over batch on 8 NeuronCores.

Reference semantics (per batch element n):
  A_flat = A.reshape(N, H, 16); h0 = c0 = mean_p(A_flat)
  xWx = x @ Wx
  per step t:
    scores[p] = (h . A_flat[:, p]) / sqrt(H)
    w = softmax(scores); attn = A_flat @ w
    a = xWx_t + h @ Wh + attn @ Wattn + b
    i,f,o,g = sig/sig/sig/tanh of quarters; c = f*c + i*g; h = o*tanh(c)
  out[:, t, :] = h

Shapes: N=512, T=64, D=512, H=512 (4H=2048). 8 cores, 64 batch each.

Per core, the 64-element batch is split into TWO independent 32-element
STREAMS, phase-staggered so that while one stream's serial
scores->softmax->attention chain runs on Vector/Scalar, the other
stream's dense GEMM work keeps the PE array continuously busy (the HAM
clock gate re-throttles the PE to 1.2 GHz after idle windows, so
sustained PE activity is worth 2x in clock alone).

Every N=512 matmul is a 4-way column-tiled QUAD: the same [128,32]
stationary is loaded at array column offsets 0/32/64/96, and the four
concurrent streams carry the four E-chunks (i,f,g,o) of the fused
weight, all accumulating into the SAME psum bank at the same column
offsets (complementary partition ranges merge in the drain). Each
partition-range's accumulation group is complete on its own (has_written
clears are per-written-element, verified on HW), so no combine step is
needed. Gates are read from their partition quarter and written to
partitions 0-31 by ScalarE's cross-partition ACT (verified on HW).

  - scores: X[m,(p,n)] quad -> cross-partition mask-mul -> grouped reduce.
  - softmax kept on the sigmoid ACT table: e^x = sig(x)/(1-sig(x)).
  - wS transpose via single DVE 32x32 stream-transpose (no PE/PSUM trip).
  - attn: block-diag matmul, stationary A_PT, moving wBD 8-col blocks.
  - state update (f*c, i*g, +) on GpSimd, freeing Vector for the chains.
"""

import math
import sys

sys.path.insert(0, "/opt/trn_rl_repo")

import numpy as np
import ml_dtypes

import concourse.bass as bass
import concourse.mybir as mybir
from concourse.tile import TileContext
from concourse.bass_utils import run_bass_kernel_spmd

N, T, D, H = 512, 64, 512, 512
E = 4 * H  # 2048
NCORES = 8
NL = N // NCORES  # 64 batch per core
B = 32  # batch per stream
P16 = 16  # attention positions
NB = 4  # batch blocks of 8 per stream
SCALE = 1.0 / math.sqrt(H)

F32 = mybir.dt.float32
MM_DT = mybir.dt.bfloat16

# E-chunk quarters [i|f|o|g] and their quad column positions
CH = {"i": 0, "f": 1, "o": 2, "g": 3}
POS = {"i": 0, "f": 32, "g": 64, "o": 96}  # psum row offset = col position


def build_nc(reps=1):
    nc = bass.Bass("TRN2", target_bir_lowering=False)

    # --- DRAM I/O ---
    xT_d = nc.declare_dram_parameter("xT", [T, D, NL], MM_DT, isOutput=False)
    AhT_d = nc.declare_dram_parameter("AhT", [H, 2, P16 * B], MM_DT, isOutput=False)
    APT_d = nc.declare_dram_parameter("APT", [128, 2, NB, H], MM_DT, isOutput=False)
    W_d = nc.declare_dram_parameter("W", [3 * H, E], MM_DT, isOutput=False)
    b_d = nc.declare_dram_parameter("bias", [1, E], MM_DT, isOutput=False)
    h0_d = nc.declare_dram_parameter("h0", [NL, H], F32, isOutput=False)
    h0T_d = nc.declare_dram_parameter("h0T", [H, NL], MM_DT, isOutput=False)
    i32_d = nc.declare_dram_parameter("i32", [B, B], F32, isOutput=False)
    d16_d = nc.declare_dram_parameter("d16", [P16, 128], MM_DT, isOutput=False)
    mPN_d = nc.declare_dram_parameter("mPN", [128, 4 * B], F32, isOutput=False)
    mBD_d = nc.declare_dram_parameter("mBD", [128, B], MM_DT, isOutput=False)
    ones1_d = nc.declare_dram_parameter("ones1", [1, B], MM_DT, isOutput=False)
    out_d = nc.declare_dram_parameter("out", [NL, T, H], F32, isOutput=True)

    Sig = mybir.ActivationFunctionType.Sigmoid
    Tanh = mybir.ActivationFunctionType.Tanh

    with TileContext(nc) as tc:
        with (
            tc.tile_pool(name="wpool", bufs=1) as wpool,
            tc.tile_pool(name="state", bufs=1) as state,
            tc.tile_pool(name="xin", bufs=3) as xin,
            tc.tile_pool(name="work0", bufs=2) as work0,
            tc.tile_pool(name="work1", bufs=2) as work1,
            tc.tile_pool(name="psA", bufs=1, space="PSUM") as psA,
            tc.tile_pool(name="psB", bufs=1, space="PSUM") as psB,
        ):
            works = (work0, work1)
            # ---- persistent SBUF tensors (shared) ----
            W_sb = wpool.tile([128, 12, E], MM_DT, tag="W")
            nc.sync.dma_start(
                out=W_sb[:], in_=W_d.ap().rearrange("(k p) e -> p k e", p=128)
            )
            b_sb = wpool.tile([1, E], MM_DT, tag="bias")
            nc.sync.dma_start(out=b_sb[:], in_=b_d[:])
            AhT_sb = wpool.tile([128, 4, 2, P16 * B], MM_DT, tag="AhT")
            nc.sync.dma_start(
                out=AhT_sb[:], in_=AhT_d.ap().rearrange("(k p) s f -> p k s f", p=128)
            )
            APT_sb = wpool.tile([128, 2, NB, H], MM_DT, tag="APT")
            nc.sync.dma_start(out=APT_sb[:], in_=APT_d[:])
            i32_sb = wpool.tile([B, B], F32, tag="i32")
            nc.sync.dma_start(out=i32_sb[:], in_=i32_d[:])
            d16_sb = wpool.tile([P16, 128], MM_DT, tag="d16")
            nc.sync.dma_start(out=d16_sb[:], in_=d16_d[:])
            mPN_sb = wpool.tile([128, 4 * B], F32, tag="mPN")
            nc.sync.dma_start(out=mPN_sb[:], in_=mPN_d[:])
            mBD_sb = wpool.tile([128, B], MM_DT, tag="mBD")
            nc.sync.dma_start(out=mBD_sb[:], in_=mBD_d[:])
            ones1_sb = wpool.tile([1, B], MM_DT, tag="ones1")
            nc.sync.dma_start(out=ones1_sb[:], in_=ones1_d[:])

            # ---- per-stream state ----
            c_sb = [None, None]
            hT_sb = [None, None]
            for s in range(2):
                c_sb[s] = state.tile([B, H], F32, tag=f"c{s}", name=f"c{s}")
                nc.sync.dma_start(out=c_sb[s][:], in_=h0_d[s * B : (s + 1) * B, :])
                hT_sb[s] = state.tile([128, 4, B], MM_DT, tag=f"hT{s}", name=f"hT{s}")
                nc.sync.dma_start(
                    out=hT_sb[s][:],
                    in_=h0T_d[:, s * B : (s + 1) * B].rearrange(
                        "(k p) n -> p k n", p=128
                    ),
                )

            # ---- PSUM: per-stream gemm bank, X bank, misc bank ----
            # ms layout (fp32 words per partition): rep [0:32), hNT j at
            # [32+32j : 64+32j), at j at [160+32j : 192+32j). 288 words < 512.
            gm_ps = [psA.tile([128, 512], F32, tag=f"gm{s}", name=f"gm{s}") for s in range(2)]
            X_ps = [psA.tile([128, 512], F32, tag=f"X{s}", name=f"Xps{s}") for s in range(2)]
            ms_ps = [psB.tile([128, 512], F32, tag=f"ms{s}", name=f"ms{s}") for s in range(2)]

            def gemm_quad(s, k, stat, start, stop):
                """One K-tile for all four chunks, 4-way column-tiled into
                stream s's gemm bank (same 512-col window -> drains merge)."""
                for cn, cpos in POS.items():
                    cs = slice(CH[cn] * 512, (CH[cn] + 1) * 512)
                    nc.tensor.matmul(
                        gm_ps[s][cpos : cpos + B, :], stat, W_sb[:, k, cs],
                        start=start, stop=stop, tile_position=(0, cpos),
                    )

            def bias_quad(s):
                for cn, cpos in POS.items():
                    cs = slice(CH[cn] * 512, (CH[cn] + 1) * 512)
                    nc.tensor.matmul(
                        gm_ps[s][cpos : cpos + B, :], ones1_sb[:], b_sb[:, cs],
                        start=True, stop=False, tile_position=(0, cpos),
                    )

            def scores_quad(s, j):
                # X[m, (p, n)] quarters (128 cols each) at row quarters
                for q in range(4):
                    nc.tensor.matmul(
                        X_ps[s][q * B : (q + 1) * B, 0:128],
                        hT_sb[s][:, j],
                        AhT_sb[:, j, s, q * 128 : (q + 1) * 128],
                        start=(j == 0), stop=(j == 3),
                        tile_position=(0, q * B),
                    )

            def seg_a2x(ss, t, xt):
                """Chain-free GEMM head: bias + xT K-tiles (covers the other
                work's serial chains)."""
                bias_quad(ss)
                for k in range(4):
                    gemm_quad(ss, k, xt[:, k, ss * B : (ss + 1) * B],
                              start=False, stop=False)

            def seg_a1h(ss, t):
                """scores + hT K-tiles + the softmax chain (DVE/ACT)."""
                work = works[ss]
                for j in range(4):
                    scores_quad(ss, j)
                for k in range(4):
                    gemm_quad(ss, 4 + k, hT_sb[ss][:, k], start=False, stop=False)

                Xm = work.tile([B, P16 * B], F32, tag="Xm")
                for q in range(4):
                    nc.vector.tensor_mul(
                        Xm[:, q * 128 : (q + 1) * 128],
                        X_ps[ss][q * B : (q + 1) * B, 0:128],
                        mPN_sb[q * B : (q + 1) * B, :],
                    )
                scS = work.tile([B, P16], F32, tag="scS")
                nc.vector.reduce_sum(
                    scS[:],
                    Xm[:].rearrange("q (p n) -> q p n", p=P16),
                    axis=mybir.AxisListType.X,
                )
                sg = work.tile([B, P16], F32, tag="sg")
                nc.scalar.activation(sg[:], scS[:], Sig, scale=SCALE)
                om = work.tile([B, P16], F32, tag="om")
                nc.vector.tensor_scalar(
                    om[:], sg[:], -1.0, 1.0,
                    op0=mybir.AluOpType.mult, op1=mybir.AluOpType.add,
                )
                omr = work.tile([B, P16], F32, tag="omr")
                nc.vector.reciprocal(omr[:], om[:])
                expS = work.tile([B, P16], F32, tag="expS")
                nc.vector.tensor_mul(expS[:], sg[:], omr[:])
                den = work.tile([B, 1], F32, tag="den")
                nc.vector.reduce_sum(den[:], expS[:], axis=mybir.AxisListType.X)
                rd = work.tile([B, 1], F32, tag="rd")
                nc.vector.reciprocal(rd[:], den[:])
                wS = work.tile([B, B], MM_DT, tag="wS")
                nc.vector.tensor_scalar_mul(wS[:, 0:P16], expS[:], rd[:])
                wST = work.tile([B, B], MM_DT, tag="wST")
                nc.vector.transpose(wST[:], wS[:])
                return wST

            def seg_b1(ss, t, wST):
                """Replicate weights, block-diag attention, attn GEMM K-tiles,
                then the gate/state chain on ACT/DVE."""
                work = works[ss]
                rep_ps = ms_ps[ss][:, 0:B]
                nc.tensor.matmul(
                    rep_ps, d16_sb[:], wST[0:P16, :], start=True, stop=True
                )
                wBD = work.tile([128, B], MM_DT, tag="wBD")
                nc.vector.tensor_mul(wBD[:], rep_ps, mBD_sb[:])

                attnT = work.tile([128, 4, B], MM_DT, tag="attnT")
                for j in range(4):
                    at_j = ms_ps[ss][:, 5 * B + j * B : 5 * B + (j + 1) * B]
                    for bb in range(NB):
                        nc.tensor.matmul(
                            at_j[:, bb * 8 : (bb + 1) * 8],
                            APT_sb[:, ss, bb, j * 128 : (j + 1) * 128],
                            wBD[:, bb * 8 : (bb + 1) * 8],
                            start=True,
                            stop=True,
                        )
                    nc.vector.tensor_copy(attnT[:, j], at_j)

                for k in range(8, 12):
                    gemm_quad(ss, k, attnT[:, k - 8], start=False, stop=(k == 11))

                ig = work.tile([B, H], F32, tag="ig")
                fg = work.tile([B, H], F32, tag="fg")
                gg = work.tile([B, H], F32, tag="gg")
                og = work.tile([B, H], F32, tag="og")
                nc.scalar.activation(
                    ig[:], gm_ps[ss][POS["i"] : POS["i"] + B, :], Sig
                )
                nc.scalar.activation(
                    fg[:], gm_ps[ss][POS["f"] : POS["f"] + B, :], Sig
                )
                fcp = work.tile([B, H], F32, tag="fcp")
                nc.vector.tensor_mul(fcp[:], fg[:], c_sb[ss][:])
                nc.scalar.activation(
                    gg[:], gm_ps[ss][POS["g"] : POS["g"] + B, :], Tanh
                )
                nc.scalar.activation(
                    og[:], gm_ps[ss][POS["o"] : POS["o"] + B, :], Sig
                )
                igp = work.tile([B, H], F32, tag="igp")
                nc.vector.tensor_mul(igp[:], ig[:], gg[:])
                nc.vector.tensor_add(c_sb[ss][:], fcp[:], igp[:])
                tc_sb = work.tile([B, H], F32, tag="tc")
                hN = work.tile([B, H], F32, tag="hN")
                for j in range(4):
                    js = slice(j * 128, (j + 1) * 128)
                    nc.scalar.activation(tc_sb[:, js], c_sb[ss][:, js], Tanh)
                    nc.vector.tensor_mul(hN[:, js], og[:, js], tc_sb[:, js])
                nc.sync.dma_start(
                    out=out_d[ss * B : (ss + 1) * B, t, :], in_=hN[:]
                )
                return hN

            def seg_b2(ss, t, hN):
                """h transposes into hT (PE + ScalarE), gated on the state
                chain; last step skips them."""
                if t >= T - 1:
                    return
                for j in range(4):
                    js = slice(j * 128, (j + 1) * 128)
                    tp_ps = ms_ps[ss][:, B + j * B : B + (j + 1) * B]
                    nc.tensor.transpose(tp_ps, hN[:, js], i32_sb[:])
                    nc.scalar.copy(hT_sb[ss][:, j], tp_ps)

            _lp = tc.For_i(0, reps, 1) if reps > 1 else None
            if _lp is not None:
                _lp.__enter__()

            xts = {}
            xt0 = xin.tile([128, 4, NL], MM_DT, tag="xT")
            xts[0] = xt0
            nc.sync.dma_start(
                out=xt0[:], in_=xT_d[0].rearrange("(k p) n -> p k n", p=128)
            )

            # Half-step slot pipeline over streams sa = u%2 / sb = 1-sa:
            #   slot u: A2x(sa, u//2) | B2(sa, u//2 - 1) | A1h(sa, u//2)
            #           | B1(sb, (u-1)//2)
            # This static per-engine order staggers the two streams so each
            # stream's serial softmax / gate chains run under the other
            # stream's (and its own chain-free) PE work.
            wst, hns = {}, {}
            for u in range(2 * T + 2):
                sa = u % 2
                ta = u // 2
                sb = 1 - sa
                tb = (u - 1) // 2
                if ta < T:
                    if ta + 1 < T and (ta + 1) not in xts and sa == 1:
                        xtn = xin.tile([128, 4, NL], MM_DT, tag="xT")
                        xts[ta + 1] = xtn
                        nc.sync.dma_start(
                            out=xtn[:],
                            in_=xT_d[ta + 1].rearrange("(k p) n -> p k n", p=128),
                        )
                    seg_a2x(sa, ta, xts[ta])
                if ta - 1 >= 0 and (sa, ta - 1) in hns:
                    seg_b2(sa, ta - 1, hns.pop((sa, ta - 1)))
                if ta < T:
                    wst[(sa, ta)] = seg_a1h(sa, ta)
                if u >= 1 and tb < T and (sb, tb) in wst:
                    hns[(sb, tb)] = seg_b1(sb, tb, wst.pop((sb, tb)))

            if _lp is not None:
                _lp.__exit__(None, None, None)

    _split_matmul_waits(nc)
    return nc


def _split_matmul_waits(nc):
    """Several TPB instruction encodings accept only one sync-wait command;
    hoist excess waits onto an inserted same-engine drain."""
    cnt = 0
    for f in nc.m.functions:
        for blk in f.blocks:
            new_insts = []
            for ins in blk.instructions:
                if (
                    ins.sync_info is not None
                    and ins.sync_info.on_wait
                    and len(ins.sync_info.on_wait) > 1
                ):
                    waits = list(ins.sync_info.on_wait)
                    for w in waits[:-1]:
                        cnt += 1
                        d = mybir.InstDrain(
                            name=f"I-mmw{cnt}", ins=[], outs=[],
                            engine=ins.engine,
                        )
                        d.sync_info = mybir.SyncInfo(on_wait=[w], on_update=[])
                        new_insts.append(d)
                    ins.sync_info = mybir.SyncInfo(
                        on_wait=[waits[-1]], on_update=list(ins.sync_info.on_update or [])
                    )
                new_insts.append(ins)
            blk.instructions = new_insts


def _prep_core_inputs(x_i, A_i, Wx, Wh, Wattn, b):
    """Host-side layout prep for one core's shard (x_i: (64,T,D), A_i: (64,H,4,4))."""
    nl = x_i.shape[0]
    A_flat = A_i.reshape(nl, H, P16)
    h0 = A_flat.mean(axis=2).astype(np.float32)  # (64, H)

    xT = np.ascontiguousarray(x_i.transpose(1, 2, 0)).astype(np.float32)  # (T, D, 64)
    # AhT[h, s, p*32+n] = A_flat[32s+n, h, p]
    AhT = np.ascontiguousarray(
        A_flat.transpose(1, 2, 0).reshape(H, P16, 2, B).transpose(0, 2, 1, 3)
        .reshape(H, 2, P16 * B)
    ).astype(np.float32)
    # APT[p*8+r, s, b, h] = A_flat[32s + 8b + r, h, p]
    APT = np.ascontiguousarray(
        A_flat.reshape(2, NB, 8, H, P16).transpose(4, 2, 0, 1, 3)
        .reshape(128, 2, NB, H)
    ).astype(np.float32)
    W = np.concatenate([Wx, Wh, Wattn], axis=0).astype(np.float32)  # (1536, E)
    i32 = np.eye(B, dtype=np.float32)
    d16 = np.repeat(np.eye(P16, dtype=np.float32), 8, axis=1)  # (16, 128)
    # mPN[q*32+m, p_local*32+n] = (n == m)
    mPN = np.tile(np.tile(np.eye(B, dtype=np.float32), (1, 4)), (4, 1))  # (128, 128)
    mBD = np.tile(np.tile(np.eye(8, dtype=np.float32), (1, NB)), (P16, 1))  # (128,32)
    ones1 = np.ones((1, B), dtype=np.float32)
    bf16 = ml_dtypes.bfloat16
    return {
        "xT": xT.astype(bf16),
        "AhT": AhT.astype(bf16),
        "APT": APT.astype(bf16),
        "W": W.astype(bf16),
        "bias": b.reshape(1, E).astype(bf16),
        "h0": h0,
        "h0T": np.ascontiguousarray(h0.T).astype(bf16),
        "i32": i32,
        "d16": d16.astype(bf16),
        "mPN": mPN,
        "mBD": mBD.astype(bf16),
        "ones1": ones1.astype(bf16),
    }


_NC_CACHE = {}


def kernel(x, A, Wx, Wh, Wattn, b, _trace=False):
    x = np.asarray(x, dtype=np.float32)
    A = np.asarray(A, dtype=np.float32)
    Wx = np.asarray(Wx, dtype=np.float32)
    Wh = np.asarray(Wh, dtype=np.float32)
    Wattn = np.asarray(Wattn, dtype=np.float32)
    b = np.asarray(b, dtype=np.float32)

    if "nc" not in _NC_CACHE:
        _NC_CACHE["nc"] = build_nc()
    nc = _NC_CACHE["nc"]

    in_maps = []
    for i in range(NCORES):
        sl = slice(i * NL, (i + 1) * NL)
        in_maps.append(_prep_core_inputs(x[sl], A[sl], Wx, Wh, Wattn, b))

    res = run_bass_kernel_spmd(
        nc, in_maps, core_ids=list(range(NCORES)), trace=_trace
    )
    outs = [res.results[i]["out"] for i in range(NCORES)]
    full = np.concatenate(outs, axis=0)  # (N, T, H)
    if _trace:
        kernel.last_exec_time_ns = res.exec_time_ns
        kernel.last_profile = res.profile_json
    return full


kernel.last_exec_time_ns = None
kernel.last_profile = None



# revision 12
# speedup vs baseline: 2.0824x; 2.0824x over previous
"""AttentionLSTM Trainium2 kernel: data-parallel over batch on 8 NeuronCores.

Reference semantics (per batch element n):
  A_flat = A.reshape(N, H, 16); h0 = c0 = mean_p(A_flat)
  xWx = x @ Wx
  per step t:
    scores[p] = (h . A_flat[:, p]) / sqrt(H)
    w = softmax(scores); attn = A_flat @ w
    a = xWx_t + h @ Wh + attn @ Wattn + b
    i,f,o,g = sig/sig/sig/tanh of quarters; c = f*c + i*g; h = o*tanh(c)
  out[:, t, :] = h

Shapes: N=512, T=64, D=512, H=512 (4H=2048). 8 cores, 64 batch each.

Per core, 64 batch = two phase-staggered 32-element streams so one
stream's serial softmax/gate chains hide under the other's dense PE
work.

v2 design (vs baseline):
  - gm PSUM double-buffered per stream: step t+1's GEMM quads never
    wait on step t's gate ACT reads -> PE never idles a MID window ->
    HAM stays at K=8/8 instead of rethrottling every slot.
  - gate layout [128 x 512] with rows (hq*32+n), cols (gate,h128) via a
    host-side column permutation of W. Every gate/state elementwise op
    runs [128 x 128-384] instead of [32 x 512]: ~2-4x less V/S time.
  - softmax on the [128,4] q-blocked scores: one mask-mul + one grouped
    reduce (was 4+1), direct Exp ACT with accum_out giving the partial
    softmax denominator for free; cross-q denominator via a tiny
    replicating PE matmul (mQQ); normalization on ScalarE via per-
    partition scale.
  - single [128,128] CASTs for attnT and hT (were 4x each).
  - bf16 hN transposes (fp32 PE transpose is a 4-pass LOW_HIGH).
"""

import math
import sys

sys.path.insert(0, "/opt/trn_rl_repo")

import numpy as np
import ml_dtypes

import concourse.bass as bass
import concourse.mybir as mybir
from concourse.tile import TileContext
from concourse.bass_utils import run_bass_kernel_spmd

N, T, D, H = 512, 64, 512, 512
E = 4 * H  # 2048
NCORES = 8
NL = N // NCORES  # 64 batch per core
B = 32  # batch per stream
P16 = 16  # attention positions
NB = 4  # batch blocks of 8 per stream
SCALE = 1.0 / math.sqrt(H)

F32 = mybir.dt.float32
BF16 = mybir.dt.bfloat16


def build_nc(split_waits=True):
    nc = bass.Bass("TRN2", target_bir_lowering=False)

    # --- DRAM I/O ---
    xT_d = nc.declare_dram_parameter("xT", [T, D, NL], BF16, isOutput=False)
    AhT_d = nc.declare_dram_parameter("AhT", [H, 2, P16 * B], BF16, isOutput=False)
    APT_d = nc.declare_dram_parameter("APT", [128, 2, NB, H], BF16, isOutput=False)
    W2_d = nc.declare_dram_parameter("W2", [3 * H, E], BF16, isOutput=False)
    b2_d = nc.declare_dram_parameter("b2", [1, E], BF16, isOutput=False)
    c0_d = nc.declare_dram_parameter("c0", [2, 128, 128], F32, isOutput=False)
    h0T_d = nc.declare_dram_parameter("h0T", [H, NL], BF16, isOutput=False)
    i128_d = nc.declare_dram_parameter("i128", [128, 128], BF16, isOutput=False)
    mPN_d = nc.declare_dram_parameter("mPN", [128, 128], F32, isOutput=False)
    mBD_d = nc.declare_dram_parameter("mBD", [128, B], BF16, isOutput=False)
    d16x_d = nc.declare_dram_parameter("d16x", [128, 128], BF16, isOutput=False)
    mQQ_d = nc.declare_dram_parameter("mQQ", [128, 128], F32, isOutput=False)
    ones1_d = nc.declare_dram_parameter("ones1", [1, B], BF16, isOutput=False)
    out_d = nc.declare_dram_parameter("out", [NL, T, H], F32, isOutput=True)

    Sig = mybir.ActivationFunctionType.Sigmoid
    Tanh = mybir.ActivationFunctionType.Tanh
    Exp = mybir.ActivationFunctionType.Exp
    Copy = mybir.ActivationFunctionType.Copy

    with TileContext(nc) as tc:
        with (
            tc.tile_pool(name="wpool", bufs=1) as wpool,
            tc.tile_pool(name="state", bufs=1) as state,
            tc.tile_pool(name="xin", bufs=3) as xin,
            tc.tile_pool(name="work0", bufs=2) as work0,
            tc.tile_pool(name="work1", bufs=2) as work1,
            tc.tile_pool(name="psG", bufs=1, space="PSUM") as psG,
            tc.tile_pool(name="psM", bufs=1, space="PSUM") as psM,
        ):
            works = (work0, work1)
            # ---- persistent SBUF tensors (shared) ----
            W2_sb = wpool.tile([128, 12, E], BF16, tag="W2")
            nc.sync.dma_start(
                out=W2_sb[:], in_=W2_d.ap().rearrange("(k p) e -> p k e", p=128)
            )
            b2_sb = wpool.tile([1, E], BF16, tag="b2")
            nc.sync.dma_start(out=b2_sb[:], in_=b2_d[:])
            AhT_sb = wpool.tile([128, 4, 2, P16 * B], BF16, tag="AhT")
            nc.sync.dma_start(
                out=AhT_sb[:], in_=AhT_d.ap().rearrange("(k p) s f -> p k s f", p=128)
            )
            APT_sb = wpool.tile([128, 2, NB, H], BF16, tag="APT")
            nc.sync.dma_start(out=APT_sb[:], in_=APT_d[:])
            i128_sb = wpool.tile([128, 128], BF16, tag="i128")
            nc.sync.dma_start(out=i128_sb[:], in_=i128_d[:])
            mPN_sb = wpool.tile([128, 128], F32, tag="mPN")
            nc.sync.dma_start(out=mPN_sb[:], in_=mPN_d[:])
            mBD_sb = wpool.tile([128, B], BF16, tag="mBD")
            nc.sync.dma_start(out=mBD_sb[:], in_=mBD_d[:])
            d16x_sb = wpool.tile([128, 128], BF16, tag="d16x")
            nc.sync.dma_start(out=d16x_sb[:], in_=d16x_d[:])
            mQQ_sb = wpool.tile([128, 128], F32, tag="mQQ")
            nc.sync.dma_start(out=mQQ_sb[:], in_=mQQ_d[:])
            ones1_sb = wpool.tile([1, B], BF16, tag="ones1")
            nc.sync.dma_start(out=ones1_sb[:], in_=ones1_d[:])

            # ---- per-stream persistent state ----
            c_sb, hT_sb, wSn_sb, wSTs_sb, r128_sb, rd128_sb = (
                [None, None] for _ in range(6)
            )
            for s in range(2):
                c_sb[s] = state.tile([128, 128], F32, tag=f"c{s}", name=f"c{s}")
                nc.sync.dma_start(out=c_sb[s][:], in_=c0_d[s])
                hT_sb[s] = state.tile([128, 4, B], BF16, tag=f"hT{s}", name=f"hT{s}")
                nc.sync.dma_start(
                    out=hT_sb[s][:],
                    in_=h0T_d[:, s * B : (s + 1) * B].rearrange(
                        "(k p) n -> p k n", p=128
                    ),
                )
                wSn_sb[s] = state.tile([128, B], BF16, tag=f"wSn{s}", name=f"wSn{s}")
                nc.vector.memset(wSn_sb[s][:], 0.0)
                wSTs_sb[s] = state.tile([128, B], BF16, tag=f"wSTs{s}", name=f"wSTs{s}")
                r128_sb[s] = state.tile([128, 1], F32, tag=f"r128_{s}", name=f"r128_{s}")
                rd128_sb[s] = state.tile([128, 1], F32, tag=f"rd128_{s}", name=f"rd128_{s}")

            # ---- PSUM ----
            # gm double-buffered per stream: 4 banks.
            gm_ps = [
                [
                    psG.tile([128, 512], F32, tag=f"gm{s}{p}", name=f"gm{s}{p}")
                    for p in range(2)
                ]
                for s in range(2)
            ]
            # misc bank per stream (f32 words): X scores at [0:128),
            # at [128:256), wBD [256:288), den128 [288:289).
            ms_ps = [psM.tile([128, 512], F32, tag=f"ms{s}", name=f"ms{s}") for s in range(2)]
            # bf16 transpose target per stream
            tp_ps = [psM.tile([128, 128], BF16, tag=f"tp{s}", name=f"tp{s}") for s in range(2)]

            def gemm_quad(s, par, k, stat, start, stop):
                """One K-tile of the fused GEMM for all four hq row-groups.
                gm rows (hq*32+n), cols (gate,h128) -- W2 is column-permuted
                so member hq streams the contiguous hq-th 512-chunk."""
                for hq in range(4):
                    nc.tensor.matmul(
                        gm_ps[s][par][hq * B : (hq + 1) * B, :],
                        stat,
                        W2_sb[:, k, hq * 512 : (hq + 1) * 512],
                        start=start,
                        stop=stop,
                        skip_group_check=True,
                        tile_position=(0, hq * B),
                    )

            def bias_quad(s, par):
                for hq in range(4):
                    nc.tensor.matmul(
                        gm_ps[s][par][hq * B : (hq + 1) * B, :],
                        ones1_sb[:],
                        b2_sb[:, hq * 512 : (hq + 1) * 512],
                        start=True,
                        stop=False,
                        skip_group_check=True,
                        tile_position=(0, hq * B),
                    )

            def seg_a2x(s, t, xt):
                """Chain-free GEMM head: bias + xT K-tiles."""
                par = t % 2
                bias_quad(s, par)
                for k in range(4):
                    gemm_quad(s, par, k, xt[:, k, s * B : (s + 1) * B],
                              start=False, stop=False)

            def seg_b2(s, t, hNb):
                """hN transposes into hT (PE bf16) + one [128,128] copy."""
                if t >= T - 1:
                    return
                nc.tensor.transpose(tp_ps[s][:], hNb[:], i128_sb[:])
                nc.vector.tensor_copy(
                    hT_sb[s][:], tp_ps[s][:].rearrange("p (k n) -> p k n", n=B)
                )

            def seg_a1h(s, t):
                """scores + hT K-tiles, then mask/reduce/exp."""
                par = t % 2
                work = works[s]
                for j in range(4):
                    for q in range(4):
                        nc.tensor.matmul(
                            ms_ps[s][q * B : (q + 1) * B, 0:128],
                            hT_sb[s][:, j],
                            AhT_sb[:, j, s, q * 128 : (q + 1) * 128],
                            start=(j == 0),
                            stop=(j == 3),
                            skip_group_check=True,
                            tile_position=(0, q * B),
                        )
                for k in range(4):
                    gemm_quad(s, par, 4 + k, hT_sb[s][:, k], start=False, stop=False)

                Xm = work.tile([128, 128], F32, tag="Xm")
                nc.vector.tensor_mul(Xm[:], ms_ps[s][:, 0:128], mPN_sb[:])
                scSq = work.tile([128, 4], F32, tag="scSq")
                nc.vector.reduce_sum(
                    scSq[:],
                    Xm[:].rearrange("p (pl n) -> p pl n", n=B),
                    axis=mybir.AxisListType.X,
                )
                expS = work.tile([128, 4], F32, tag="expS")
                nc.scalar.activation(
                    expS[:], scSq[:], Exp, scale=SCALE, accum_out=r128_sb[s][:]
                )
                return expS

            def seg_tail(s, t, expS):
                """den replication matmul + recip + normalize + transpose."""
                den = ms_ps[s][:, 288:289]
                nc.tensor.matmul(den, mQQ_sb[:], r128_sb[s][:], start=True, stop=True)
                nc.vector.reciprocal(rd128_sb[s][:], den)
                nc.scalar.activation(
                    wSn_sb[s][:, 0:4], expS[:], Copy, scale=rd128_sb[s][:]
                )
                nc.vector.transpose(wSTs_sb[s][:], wSn_sb[s][:])

            def seg_b1(s, t):
                """attention + attn GEMM K-tiles + gates/state update."""
                par = t % 2
                work = works[s]
                wBD_ps = ms_ps[s][:, 256:288]
                nc.tensor.matmul(
                    wBD_ps, d16x_sb[:], wSTs_sb[s][:], start=True, stop=True
                )
                wBDs = work.tile([128, B], BF16, tag="wBDs")
                nc.vector.tensor_mul(wBDs[:], wBD_ps, mBD_sb[:])

                for j in range(4):
                    at_j = ms_ps[s][:, 128 + j * B : 128 + (j + 1) * B]
                    for bb in range(NB):
                        nc.tensor.matmul(
                            at_j[:, bb * 8 : (bb + 1) * 8],
                            APT_sb[:, s, bb, j * 128 : (j + 1) * 128],
                            wBDs[:, bb * 8 : (bb + 1) * 8],
                            start=True,
                            stop=True,
                        )
                attnT = work.tile([128, 4, B], BF16, tag="attnT")
                nc.vector.tensor_copy(
                    attnT[:], ms_ps[s][:, 128:256].rearrange("p (k n) -> p k n", n=B)
                )

                for k in range(8, 12):
                    gemm_quad(s, par, k, attnT[:, k - 8], start=False, stop=(k == 11))

                gm = gm_ps[s][par]
                sg = work.tile([128, 384], BF16, tag="sg")
                nc.scalar.activation(sg[:], gm[:, 0:384], Sig)
                gg = work.tile([128, 128], BF16, tag="gg")
                nc.scalar.activation(gg[:], gm[:, 384:512], Tanh)
                fcp = work.tile([128, 128], F32, tag="fcp")
                nc.vector.tensor_mul(fcp[:], sg[:, 128:256], c_sb[s][:])
                igp = work.tile([128, 128], BF16, tag="igp")
                nc.vector.tensor_mul(igp[:], sg[:, 0:128], gg[:])
                nc.vector.tensor_add(c_sb[s][:], fcp[:], igp[:])
                tc_t = work.tile([128, 128], BF16, tag="tc")
                nc.scalar.activation(tc_t[:], c_sb[s][:], Tanh)
                hN = work.tile([128, 128], F32, tag="hN")
                nc.vector.tensor_mul(hN[:], sg[:, 256:384], tc_t[:])
                for hq in range(4):
                    nc.sync.dma_start(
                        out=out_d[s * B : (s + 1) * B, t, hq * 128 : (hq + 1) * 128],
                        in_=hN[hq * B : (hq + 1) * B, :],
                    )
                hNb = work.tile([128, 128], BF16, tag="hNb")
                nc.vector.tensor_copy(hNb[:], hN[:])
                return hNb

            # ---- slot pipeline over streams ----
            xts = {}
            xt0 = xin.tile([128, 4, NL], BF16, tag="xT")
            xts[0] = xt0
            nc.sync.dma_start(
                out=xt0[:], in_=xT_d[0].rearrange("(k p) n -> p k n", p=128)
            )

            exps, hnbs = {}, {}
            for u in range(2 * T + 2):
                sa = u % 2
                ta = u // 2
                sb = 1 - sa
                tb = (u - 1) // 2
                if ta < T:
                    if ta + 1 < T and (ta + 1) not in xts and sa == 1:
                        xtn = xin.tile([128, 4, NL], BF16, tag="xT")
                        xts[ta + 1] = xtn
                        nc.sync.dma_start(
                            out=xtn[:],
                            in_=xT_d[ta + 1].rearrange("(k p) n -> p k n", p=128),
                        )
                    seg_a2x(sa, ta, xts[ta])
                if ta - 1 >= 0 and (sa, ta - 1) in hnbs:
                    seg_b2(sa, ta - 1, hnbs.pop((sa, ta - 1)))
                if ta < T:
                    exps[(sa, ta)] = seg_a1h(sa, ta)
                if u >= 1 and tb < T:
                    hnbs[(sb, tb)] = seg_b1(sb, tb)
                if ta < T:
                    seg_tail(sa, ta, exps.pop((sa, ta)))

    if split_waits:
        _split_matmul_waits(nc)
    return nc


def _split_matmul_waits(nc):
    """Several TPB instruction encodings accept only one sync-wait command;
    hoist excess waits onto an inserted same-engine drain."""
    cnt = 0
    for f in nc.m.functions:
        for blk in f.blocks:
            new_insts = []
            for ins in blk.instructions:
                if (
                    ins.sync_info is not None
                    and ins.sync_info.on_wait
                    and len(ins.sync_info.on_wait) > 1
                ):
                    waits = list(ins.sync_info.on_wait)
                    for w in waits[:-1]:
                        cnt += 1
                        d = mybir.InstDrain(
                            name=f"I-mmw{cnt}", ins=[], outs=[],
                            engine=ins.engine,
                        )
                        d.sync_info = mybir.SyncInfo(on_wait=[w], on_update=[])
                        new_insts.append(d)
                    ins.sync_info = mybir.SyncInfo(
                        on_wait=[waits[-1]], on_update=list(ins.sync_info.on_update or [])
                    )
                new_insts.append(ins)
            blk.instructions = new_insts


def _prep_core_inputs(x_i, A_i, Wx, Wh, Wattn, b):
    """Host-side layout prep for one core's shard (x_i: (64,T,D), A_i: (64,H,4,4))."""
    nl = x_i.shape[0]
    A_flat = A_i.reshape(nl, H, P16)
    h0 = A_flat.mean(axis=2).astype(np.float32)  # (64, H)

    xT = np.ascontiguousarray(x_i.transpose(1, 2, 0)).astype(np.float32)  # (T, D, 64)
    # AhT[h, s, p*32+n] = A_flat[32s+n, h, p]
    AhT = np.ascontiguousarray(
        A_flat.transpose(1, 2, 0).reshape(H, P16, 2, B).transpose(0, 2, 1, 3)
        .reshape(H, 2, P16 * B)
    ).astype(np.float32)
    # APT[p*8+r, s, b, h] = A_flat[32s + 8b + r, h, p]
    APT = np.ascontiguousarray(
        A_flat.reshape(2, NB, 8, H, P16).transpose(4, 2, 0, 1, 3)
        .reshape(128, 2, NB, H)
    ).astype(np.float32)
    # W columns permuted: W2[:, hq*512 + g*128 + h1] = W[:, g*512 + hq*128 + h1]
    W = np.concatenate([Wx, Wh, Wattn], axis=0).astype(np.float32)  # (1536, E)
    W2 = np.ascontiguousarray(
        W.reshape(3 * H, 4, 4, 128).transpose(0, 2, 1, 3).reshape(3 * H, E)
    )
    b2 = np.ascontiguousarray(
        b.reshape(4, 4, 128).transpose(1, 0, 2).reshape(1, E)
    ).astype(np.float32)
    # c0[s, hq*32+n, h1] = h0[s*32+n, hq*128+h1]
    c0 = np.ascontiguousarray(
        h0.reshape(2, B, 4, 128).transpose(0, 2, 1, 3).reshape(2, 128, 128)
    )
    i128 = np.eye(128, dtype=np.float32)
    # mPN[q*32+m, pl*32+n] = (n == m)
    mPN = np.tile(np.tile(np.eye(B, dtype=np.float32), (1, 4)), (4, 1))  # (128, 128)
    mBD = np.tile(np.tile(np.eye(8, dtype=np.float32), (1, NB)), (P16, 1))  # (128,32)
    # d16x[q*32+pl, p*8+r] = (p == q*4+pl), pl<4
    d16x = np.zeros((128, 128), dtype=np.float32)
    for p in range(P16):
        q, pl = p // 4, p % 4
        d16x[q * 32 + pl, p * 8 : (p + 1) * 8] = 1.0
    mQQ = np.tile(np.eye(B, dtype=np.float32), (4, 4))  # (128, 128)
    ones1 = np.ones((1, B), dtype=np.float32)
    bf16 = ml_dtypes.bfloat16
    return {
        "xT": xT.astype(bf16),
        "AhT": AhT.astype(bf16),
        "APT": APT.astype(bf16),
        "W2": W2.astype(bf16),
        "b2": b2.astype(bf16),
        "c0": c0,
        "h0T": np.ascontiguousarray(h0.T).astype(bf16),
        "i128": i128.astype(bf16),
        "mPN": mPN,
        "mBD": mBD.astype(bf16),
        "d16x": d16x.astype(bf16),
        "mQQ": mQQ,
        "ones1": ones1.astype(bf16),
    }


_NC_CACHE = {}


def kernel(x, A, Wx, Wh, Wattn, b, _trace=False):
    x = np.asarray(x, dtype=np.float32)
    A = np.asarray(A, dtype=np.float32)
    Wx = np.asarray(Wx, dtype=np.float32)
    Wh = np.asarray(Wh, dtype=np.float32)
    Wattn = np.asarray(Wattn, dtype=np.float32)
    b = np.asarray(b, dtype=np.float32)

    if "nc" not in _NC_CACHE:
        _NC_CACHE["nc"] = build_nc()
    nc = _NC_CACHE["nc"]

    in_maps = []
    for i in range(NCORES):
        sl = slice(i * NL, (i + 1) * NL)
        in_maps.append(_prep_core_inputs(x[sl], A[sl], Wx, Wh, Wattn, b))

    res = run_bass_kernel_spmd(
        nc, in_maps, core_ids=list(range(NCORES)), trace=_trace
    )
    outs = [res.results[i]["out"] for i in range(NCORES)]
    full = np.concatenate(outs, axis=0)  # (N, T, H)
    if _trace:
        kernel.last_exec_time_ns = res.exec_time_ns
        kernel.last_profile = res.profile_json
    return full


kernel.last_exec_time_ns = None
kernel.last_profile = None
